# revision 1
# baseline (speedup 1.0000x reference)
"""Trainium2 Bass kernel for nn_Channel_Attention (XCA-style windowed channel attention).

Per core (data-parallel over batch x H-half, slab = 128 rows x 256 cols):
  - depthwise-3x3 folded into the 1x1 qkv conv: 9 shifted GEMMs accumulate in PSUM
    (host precomputes W_t = diag(dw_w[:,dy,dx]) @ qkv_w, so qkv_dw = sum_t W_t @ shift_t(x))
  - N-blocks = 8 rows x 64 cols = 8 complete 8x8 windows, so per-(chan,window) l2
    norms (ACT Square + DVE window reduce + sqrt + recip) finish within one PSUM tile
  - q,k evacuated with fused 1/norm scaling (DVE scalar_tensor_tensor, broadcast AP)
  - per-window pc layout via PE transpose; per-head 32x32 grams; exp without max
    subtraction (|dots| <= temp since q,k are l2-normalized); DVE 32x32 block
    transpose of E; out2 = E^T.T @ v; 1/Z fused into evacuation; final proj GEMM
"""
import sys

sys.path.insert(0, "/opt/trn_rl_repo")

import numpy as np

DIM, HEADS, PS = 192, 6, 8
B, H, W = 4, 256, 256
C3 = 3 * DIM  # 576
NCORES = 8
ROWS = 128
NSTRIP = 16

_CACHE = {}
LAST_RESULT = None


def _build_nc():
    import contextlib

    import concourse.mybir as mybir
    import concourse.tile as tile
    from concourse import bacc

    f32 = mybir.dt.float32
    AL = mybir.AluOpType
    AF = mybir.ActivationFunctionType
    AX = mybir.AxisListType

    nc = bacc.Bacc("TRN2", target_bir_lowering=False, debug=False, num_devices=NCORES)

    xp_e = nc.declare_dram_parameter("xp", [DIM, 130, 258], f32, isOutput=False)
    wtA_e = nc.declare_dram_parameter("wtA", [128, 9 * C3], f32, isOutput=False)
    wtB_e = nc.declare_dram_parameter("wtB", [64, 9 * C3], f32, isOutput=False)
    pjA_e = nc.declare_dram_parameter("pjA", [128, 192], f32, isOutput=False)
    pjB_e = nc.declare_dram_parameter("pjB", [64, 192], f32, isOutput=False)
    tmpA_e = nc.declare_dram_parameter("tmpA", [128, 1], f32, isOutput=False)
    tmpB_e = nc.declare_dram_parameter("tmpB", [64, 1], f32, isOutput=False)
    idn_e = nc.declare_dram_parameter("idn", [128, 128], f32, isOutput=False)
    mAl_e = nc.declare_dram_parameter("mAl", [5, 128], f32, isOutput=False)
    mAr_e = nc.declare_dram_parameter("mAr", [5, 512], f32, isOutput=False)
    mBl_e = nc.declare_dram_parameter("mBl", [3, 64], f32, isOutput=False)
    mBr_e = nc.declare_dram_parameter("mBr", [3, 256], f32, isOutput=False)
    y_e = nc.declare_dram_parameter("y", [DIM, ROWS, 256], f32, isOutput=True)

    with tile.TileContext(nc) as tc, contextlib.ExitStack() as ctx:
        const = ctx.enter_context(tc.tile_pool(name="const", bufs=1))
        xs_p = ctx.enter_context(tc.tile_pool(name="xs", bufs=2))
        qk_p = ctx.enter_context(tc.tile_pool(name="qk", bufs=1))
        pc_p = ctx.enter_context(tc.tile_pool(name="pc", bufs=1))
        sm_p = ctx.enter_context(tc.tile_pool(name="small", bufs=2))
        e_p = ctx.enter_context(tc.tile_pool(name="eb", bufs=2))
        att_p = ctx.enter_context(tc.tile_pool(name="att", bufs=1))
        y_p = ctx.enter_context(tc.tile_pool(name="yb", bufs=1))
        ps_big = ctx.enter_context(tc.tile_pool(name="psbig", bufs=3, space="PSUM"))
        ps_pct = ctx.enter_context(tc.tile_pool(name="pspct", bufs=2, space="PSUM"))
        ps_gA = ctx.enter_context(tc.tile_pool(name="psga", bufs=1, space="PSUM"))
        ps_gB = ctx.enter_context(tc.tile_pool(name="psgb", bufs=1, space="PSUM"))
        ps_oA = ctx.enter_context(tc.tile_pool(name="psoa", bufs=1, space="PSUM"))

        # ---- constants ----
        wtA = const.tile([128, 9 * C3], f32)
        nc.sync.dma_start(wtA[:], wtA_e[:, :])
        wtB = const.tile([64, 9 * C3], f32)
        nc.sync.dma_start(wtB[:], wtB_e[:, :])
        pjA = const.tile([128, 192], f32)
        nc.sync.dma_start(pjA[:], pjA_e[:, :])
        pjB = const.tile([64, 192], f32)
        nc.sync.dma_start(pjB[:], pjB_e[:, :])
        tmpA = const.tile([128, 1], f32)
        nc.sync.dma_start(tmpA[:], tmpA_e[:, :])
        tmpB = const.tile([64, 1], f32)
        nc.sync.dma_start(tmpB[:], tmpB_e[:, :])
        idn = const.tile([128, 128], f32)
        nc.sync.dma_start(idn[:], idn_e[:, :])
        mAl = const.tile([5, 128], f32)
        nc.sync.dma_start(mAl[:], mAl_e[:, :])
        mAr = const.tile([5, 512], f32)
        nc.sync.dma_start(mAr[:], mAr_e[:, :])
        mBl = const.tile([3, 64], f32)
        nc.sync.dma_start(mBl[:], mBl_e[:, :])
        mBr = const.tile([3, 256], f32)
        nc.sync.dma_start(mBr[:], mBr_e[:, :])

        MCH = [(0, 128), (128, 128), (256, 128), (384, 128), (512, 64)]

        for s in range(NSTRIP):
            # ---- load x strip (10 rows x 258 cols, padded) ----
            xsA = xs_p.tile([128, 10 * 258], f32, tag="xsA")
            nc.sync.dma_start(xsA[:], xp_e[0:128, 8 * s:8 * s + 10, :])
            xsB = xs_p.tile([64, 10 * 258], f32, tag="xsB")
            nc.sync.dma_start(xsB[:], xp_e[128:192, 8 * s:8 * s + 10, :])
            xsA3 = xsA[:].rearrange("p (r c) -> p r c", r=10)
            xsB3 = xsB[:].rearrange("p (r c) -> p r c", r=10)

            qk = [qk_p.tile([128, 2048], f32, name=f"qk{m}", tag=f"qk{m}") for m in range(3)]
            v3 = qk_p.tile([128, 2048], f32, tag="v3")
            v4 = qk_p.tile([64, 2048], f32, tag="v4")

            # ---- fold GEMM + norms + evac ----
            # N-block j = 8 rows x cols [64j, 64j+64) = windows [8j, 8j+8)
            for m, (mo, msz) in enumerate(MCH):
                for jp in range(2):
                    psl = []
                    for j in (2 * jp, 2 * jp + 1):
                        ps = ps_big.tile([128, 512], f32, tag="big")
                        psl.append((j, ps))
                    for t in range(9):
                        dy, dx = t // 3, t % 3
                        for kc in range(2):
                            wt = (wtA, wtB)[kc]
                            lhsT = wt[:, t * C3 + mo: t * C3 + mo + msz]
                            xs3 = (xsA3, xsB3)[kc]
                            for j, ps in psl:
                                rhs = xs3[:, dy:dy + 8,
                                          64 * j + dx:64 * j + dx + 64
                                          ].rearrange("p r (w c) -> p w r c", w=8)
                                nc.tensor.matmul(
                                    ps[0:msz, :], lhsT, rhs,
                                    start=(t == 0 and kc == 0),
                                    stop=(t == 8 and kc == 1),
                                )
                    for j, ps in psl:
                        if m < 3:
                            sq = sm_p.tile([128, 512], f32, tag="sq")
                            nc.scalar.activation(sq[0:msz, :], ps[0:msz, :],
                                                 AF.Square)
                            sqv = sq[0:msz, :].rearrange("p (w c) -> p w c", w=8)
                            n2 = sm_p.tile([128, 8], f32, tag="n2")
                            nc.vector.tensor_reduce(n2[0:msz, :], sqv,
                                                    axis=AX.X, op=AL.add)
                            nrm = sm_p.tile([128, 8], f32, tag="nrm")
                            nc.scalar.activation(nrm[0:msz, :], n2[0:msz, :],
                                                 AF.Sqrt)
                            rj = sm_p.tile([128, 8], f32, tag="rj")
                            nc.vector.reciprocal(rj[0:msz, :], nrm[0:msz, :])
                            # evac with 1/norm scaling (win-grouped layout)
                            dst = qk[m][:, 512 * j:512 * (j + 1)].rearrange(
                                "p (w c) -> p w c", w=8)
                            nc.vector.scalar_tensor_tensor(
                                dst, ps[0:msz, :].rearrange("p (w c) -> p w c",
                                                            w=8),
                                1.0,
                                rj[0:msz, :].unsqueeze(2)
                                .broadcast_to((msz, 8, 64)),
                                AL.mult, AL.mult)
                        else:
                            dstt = (v3, v4)[m - 3]
                            nc.scalar.copy(dstt[0:msz, 512 * j:512 * (j + 1)],
                                           ps[0:msz, :])

            # ---- pc transposes + attention, halves of 16 windows ----
            for half in range(2):
                if half == 0:
                    att_A = att_p.tile([128, 2048], f32, tag="attA")
                    att_B = att_p.tile([64, 2048], f32, tag="attB")
                qkpc = pc_p.tile([128, 8 * 384], f32, tag="qkpc")
                for p in range(8):
                    w0 = 16 * half + 2 * p
                    pcps = ps_pct.tile([128, 384], f32, tag="pct")
                    for (src, oc) in ((qk[0], 0), (qk[1], 128), (qk[2], 256)):
                        inap = src[0:128, 64 * w0:64 * w0 + 128]
                        nc.tensor.transpose(pcps[:, oc:oc + 128], inap, idn[:, :])
                    nc.vector.tensor_copy(qkpc[:, 384 * p:384 * (p + 1)], pcps[:])
                # grams + attention, per quarter of 4 windows
                for quad in range(4):
                    gA = ps_gA.tile([128, 512], f32, tag="ga")
                    gB = ps_gB.tile([64, 256], f32, tag="gb")
                    # mask bias first (start=True clears bank, writes -100 off-block)
                    nc.tensor.matmul(gA[:], mAl[:], mAr[:], start=True,
                                     stop=False, skip_group_check=True)
                    nc.tensor.matmul(gB[0:64, :], mBl[:], mBr[:], start=True,
                                     stop=False, skip_group_check=True)
                    for wq in range(4):
                        wl = 4 * quad + wq
                        p, wsub = wl // 2, wl % 2
                        pb = 64 * wsub
                        qa = qkpc[pb:pb + 64, 384 * p:384 * p + 128]
                        ka = qkpc[pb:pb + 64, 384 * p + 192:384 * p + 320]
                        nc.tensor.matmul(gA[:, 128 * wq:128 * wq + 128], qa, ka,
                                         start=False, stop=(wq == 3),
                                         skip_group_check=True)
                        qb = qkpc[pb:pb + 64, 384 * p + 128:384 * p + 192]
                        kb = qkpc[pb:pb + 64, 384 * p + 320:384 * p + 384]
                        nc.tensor.matmul(gB[0:64, 64 * wq:64 * wq + 64], qb, kb,
                                         start=False, stop=(wq == 3),
                                         skip_group_check=True)
                    eA = e_p.tile([128, 512], f32, tag="eA")
                    eB = e_p.tile([64, 256], f32, tag="eB")
                    nc.scalar.activation(eA[:], gA[:], AF.Exp, scale=tmpA[:])
                    nc.scalar.activation(eB[0:64, :], gB[0:64, :], AF.Exp,
                                         scale=tmpB[:])
                    zA = sm_p.tile([128, 4], f32, tag="zA")
                    zB = sm_p.tile([64, 4], f32, tag="zB")
                    nc.vector.tensor_reduce(
                        zA[:], eA[:].rearrange("p (w j) -> p w j", w=4),
                        axis=AX.X, op=AL.add)
                    nc.vector.tensor_reduce(
                        zB[0:64, :], eB[0:64, :].rearrange("p (w j) -> p w j",
                                                           w=4),
                        axis=AX.X, op=AL.add)
                    rzA = sm_p.tile([128, 4], f32, tag="rzA")
                    rzB = sm_p.tile([64, 4], f32, tag="rzB")
                    nc.vector.reciprocal(rzA[:], zA[:])
                    nc.vector.reciprocal(rzB[0:64, :], zB[0:64, :])
                    etA = e_p.tile([128, 512], f32, tag="etA")
                    etB = e_p.tile([64, 256], f32, tag="etB")
                    nc.vector.transpose(etA[:], eA[:])
                    nc.vector.transpose(etB[0:64, :], eB[0:64, :])
                    o2 = ps_oA.tile([128, 512], f32, tag="oa")
                    oA = o2[:, 0:256]
                    oB = o2[0:64, 256:512]
                    for wq in range(4):
                        wl = 4 * quad + wq
                        wg = 16 * half + wl
                        nc.tensor.matmul(oA[0:128, 64 * wq:64 * wq + 64],
                                         etA[:, 128 * wq:128 * wq + 128],
                                         v3[:, 64 * wg:64 * wg + 64],
                                         start=True, stop=True)
                        nc.tensor.matmul(oB[0:64, 64 * wq:64 * wq + 64],
                                         etB[0:64, 64 * wq:64 * wq + 64],
                                         v4[0:64, 64 * wg:64 * wg + 64],
                                         start=True, stop=True)
                    ob = 1024 * half + 256 * quad
                    nc.vector.scalar_tensor_tensor(
                        att_A[:, ob:ob + 256].rearrange("p (w c) -> p w c", w=4),
                        oA[0:128, :].rearrange("p (w c) -> p w c", w=4),
                        1.0,
                        rzA[:].unsqueeze(2).broadcast_to((128, 4, 64)),
                        AL.mult, AL.mult)
                    nc.vector.scalar_tensor_tensor(
                        att_B[0:64, ob:ob + 256].rearrange("p (w c) -> p w c",
                                                           w=4),
                        oB[0:64, 0:256].rearrange("p (w c) -> p w c", w=4),
                        1.0,
                        rzB[0:64, :].unsqueeze(2).broadcast_to((64, 4, 64)),
                        AL.mult, AL.mult)

            # ---- proj GEMM (restores flat pixel order) ----
            yA = y_p.tile([128, 2048], f32, tag="yA")
            yB = y_p.tile([64, 2048], f32, tag="yB")
            attAr = att_A[:, :].rearrange("p (w r c) -> p r w c", w=32, r=8)
            attBr = att_B[0:64, :].rearrange("p (w r c) -> p r w c", w=32, r=8)
            for nb in range(4):
                ppA = ps_big.tile([128, 512], f32, tag="big")
                ppB = ps_gB.tile([64, 512], f32, tag="gb")
                for mo2, msz2, pp in ((0, 128, ppA), (128, 64, ppB)):
                    for kc in range(2):
                        lhsT = (pjA, pjB)[kc][:, mo2:mo2 + msz2]
                        rhs = (attAr, attBr)[kc][:, 2 * nb:2 * nb + 2, :, :]
                        nc.tensor.matmul(pp[0:msz2, :], lhsT, rhs,
                                         start=(kc == 0), stop=(kc == 1))
                nc.vector.tensor_copy(yA[:, 512 * nb:512 * (nb + 1)], ppA[:])
                nc.scalar.copy(yB[0:64, 512 * nb:512 * (nb + 1)], ppB[0:64, :])
            nc.sync.dma_start(y_e[0:128, 8 * s:8 * s + 8, :], yA[:])
            nc.sync.dma_start(y_e[128:192, 8 * s:8 * s + 8, :], yB[0:64, :])

    nc.finalize()
    return nc


def kernel(**inputs):
    global LAST_RESULT
    from concourse.bass_utils import run_bass_kernel_spmd

    x = np.ascontiguousarray(np.asarray(inputs["x"], dtype=np.float32))
    qkv_w = np.asarray(inputs["qkv_w"], dtype=np.float32)[:, :, 0, 0]
    dw_w = np.asarray(inputs["dw_w"], dtype=np.float32)[:, 0]
    proj_w = np.asarray(inputs["proj_w"], dtype=np.float32)[:, :, 0, 0]
    temp = np.asarray(inputs["temperature"], dtype=np.float32)[:, 0, 0]

    wtT = np.stack([(qkv_w * dw_w[:, t // 3, t % 3][:, None]).T.copy()
                    for t in range(9)])  # [9, 192, 576]
    wtA = np.ascontiguousarray(
        wtT[:, 0:128, :].transpose(1, 0, 2).reshape(128, 9 * C3))
    wtB = np.ascontiguousarray(
        wtT[:, 128:192, :].transpose(1, 0, 2).reshape(64, 9 * C3))
    projT = np.ascontiguousarray(proj_w.T)
    pjA = np.ascontiguousarray(projT[0:128, :])
    pjB = np.ascontiguousarray(projT[128:192, :])
    tmpA = np.repeat(temp[0:4], 32).reshape(128, 1).astype(np.float32)
    tmpB = np.repeat(temp[4:6], 32).reshape(64, 1).astype(np.float32)
    idn = np.eye(128, dtype=np.float32)
    mAl = np.zeros((5, 128), np.float32)
    mAr = np.zeros((5, 512), np.float32)
    mAl[0, :] = 10.0
    mAr[0, :] = -10.0
    for g in range(4):
        mAl[1 + g, 32 * g:32 * g + 32] = 10.0
        for rep in range(4):
            mAr[1 + g, 128 * rep + 32 * g:128 * rep + 32 * g + 32] = 10.0
    mBl = np.zeros((3, 64), np.float32)
    mBr = np.zeros((3, 256), np.float32)
    mBl[0, :] = 10.0
    mBr[0, :] = -10.0
    for g in range(2):
        mBl[1 + g, 32 * g:32 * g + 32] = 10.0
        for rep in range(4):
            mBr[1 + g, 64 * rep + 32 * g:64 * rep + 32 * g + 32] = 10.0

    xpad = np.pad(x, ((0, 0), (0, 0), (1, 1), (1, 1)))
    in_maps = []
    for core in range(NCORES):
        b, halfc = core // 2, core % 2
        r0 = 128 * halfc
        xp = np.ascontiguousarray(xpad[b, :, r0:r0 + 130, :])
        in_maps.append(dict(xp=xp, wtA=wtA, wtB=wtB, pjA=pjA, pjB=pjB,
                            tmpA=tmpA, tmpB=tmpB, idn=idn, mAl=mAl, mAr=mAr,
                            mBl=mBl, mBr=mBr))

    if "nc" not in _CACHE:
        _CACHE["nc"] = _build_nc()
    nc = _CACHE["nc"]

    res = run_bass_kernel_spmd(nc, in_maps, list(range(NCORES)))
    LAST_RESULT = res

    out = np.zeros((B, DIM, H, W), np.float32)
    for core in range(NCORES):
        b, halfc = core // 2, core % 2
        r0 = 128 * halfc
        out[b, :, r0:r0 + 128, :] = res.results[core]["y"]
    return out



# revision 12
# speedup vs baseline: 1.2253x; 1.2253x over previous
"""Trainium2 Bass kernel for nn_Channel_Attention (XCA-style windowed channel attention).

De-folded design (per core = one batch x one H-half, slab 128 rows x 256 cols):
  - 1x1 qkv GEMM on PE (bf16, 1x cost) over a 10-row x 258-col haloed strip;
    PSUM evacuated to SBUF bf16 by ACT (halo cols/rows inherit x's zero pad).
  - depthwise 3x3 as 9 shifted multiply-accumulates on the vector engines:
    m0/m2/m4 on DVE (tensor_scalar 4x mults + tensor_tensor 2x tree adds),
    m1 mults on ACT (per-partition scale), m3 on GpSimd (fused
    scalar_tensor_tensor ping-pong chain). Weights are per-partition [P,1] APs.
  - attention with TRANSPOSED gram (lhsT=k_pc, rhs=q_pc) so E'=[j,i] feeds the
    out matmul directly (no DVE transpose); softmax denominator comes free via
    a ones-column appended to v (window stride 65). Mask bias matmuls stay
    fp32 (bf16 K<8 mask MMs hang the device).
  - proj GEMM restores raster order; fp32 y out.
"""
import sys

sys.path.insert(0, "/opt/trn_rl_repo")

import numpy as np
import ml_dtypes

BF16 = ml_dtypes.bfloat16

DIM, HEADS, PS = 192, 6, 8
B, H, W = 4, 256, 256
C3 = 3 * DIM  # 576
NCORES = 8
ROWS = 128
NSTRIP = 16

_CACHE = {}
LAST_RESULT = None

MCH = [(0, 128), (128, 128), (256, 128), (384, 128), (512, 64)]


def _build_nc():
    import contextlib

    import concourse.mybir as mybir
    import concourse.tile as tile
    from concourse import bacc

    f32 = mybir.dt.float32
    bf16 = mybir.dt.bfloat16
    AL = mybir.AluOpType
    AF = mybir.ActivationFunctionType
    AX = mybir.AxisListType

    nc = bacc.Bacc("TRN2", target_bir_lowering=False, debug=False, num_devices=NCORES)

    xp_e = nc.declare_dram_parameter("xp", [DIM, 130, 258], bf16, isOutput=False)
    w1A_e = nc.declare_dram_parameter("w1A", [128, C3], bf16, isOutput=False)
    w1B_e = nc.declare_dram_parameter("w1B", [64, C3], bf16, isOutput=False)
    dwW_e = nc.declare_dram_parameter("dwW", [128, 45], f32, isOutput=False)
    pjA_e = nc.declare_dram_parameter("pjA", [128, 192], bf16, isOutput=False)
    pjB_e = nc.declare_dram_parameter("pjB", [64, 192], bf16, isOutput=False)
    tmpA_e = nc.declare_dram_parameter("tmpA", [128, 1], f32, isOutput=False)
    tmpB_e = nc.declare_dram_parameter("tmpB", [64, 1], f32, isOutput=False)
    idn_e = nc.declare_dram_parameter("idn", [128, 128], bf16, isOutput=False)
    mAl_e = nc.declare_dram_parameter("mAl", [5, 128], f32, isOutput=False)
    mAr_e = nc.declare_dram_parameter("mAr", [5, 512], f32, isOutput=False)
    mBl_e = nc.declare_dram_parameter("mBl", [3, 64], f32, isOutput=False)
    mBr_e = nc.declare_dram_parameter("mBr", [3, 256], f32, isOutput=False)
    y_e = nc.declare_dram_parameter("y", [DIM, ROWS, 256], f32, isOutput=True)

    with tile.TileContext(nc) as tc, contextlib.ExitStack() as ctx:
        const = ctx.enter_context(tc.tile_pool(name="const", bufs=1))
        xs_p = ctx.enter_context(tc.tile_pool(name="xs", bufs=2))
        q1_p = ctx.enter_context(tc.tile_pool(name="q1", bufs=1))
        tap_p = ctx.enter_context(tc.tile_pool(name="tap", bufs=1))
        acc_p = ctx.enter_context(tc.tile_pool(name="acc", bufs=1))
        qk_p = ctx.enter_context(tc.tile_pool(name="qk", bufs=1))
        sq_p = ctx.enter_context(tc.tile_pool(name="sq", bufs=1))
        sm_p = ctx.enter_context(tc.tile_pool(name="small", bufs=2))
        pc_p = ctx.enter_context(tc.tile_pool(name="pc", bufs=1))
        e_p = ctx.enter_context(tc.tile_pool(name="eb", bufs=2))
        att_p = ctx.enter_context(tc.tile_pool(name="att", bufs=1))
        y_p = ctx.enter_context(tc.tile_pool(name="yb", bufs=1))
        pmm = ctx.enter_context(tc.tile_pool(name="pmm", bufs=2, space="PSUM"))
        pct = ctx.enter_context(tc.tile_pool(name="pct", bufs=1, space="PSUM"))
        pg = ctx.enter_context(tc.tile_pool(name="pg", bufs=1, space="PSUM"))
        pgB = ctx.enter_context(tc.tile_pool(name="pgB", bufs=1, space="PSUM"))
        po = ctx.enter_context(tc.tile_pool(name="po", bufs=1, space="PSUM"))

        # ---- constants ----
        w1A = const.tile([128, C3], bf16)
        nc.sync.dma_start(w1A[:], w1A_e[:, :])
        w1B = const.tile([64, C3], bf16)
        nc.sync.dma_start(w1B[:], w1B_e[:, :])
        dwW = const.tile([128, 45], f32)
        nc.sync.dma_start(dwW[:], dwW_e[:, :])
        pjA = const.tile([128, 192], bf16)
        nc.sync.dma_start(pjA[:], pjA_e[:, :])
        pjB = const.tile([64, 192], bf16)
        nc.sync.dma_start(pjB[:], pjB_e[:, :])
        tmpA = const.tile([128, 1], f32)
        nc.sync.dma_start(tmpA[:], tmpA_e[:, :])
        tmpB = const.tile([64, 1], f32)
        nc.sync.dma_start(tmpB[:], tmpB_e[:, :])
        idn = const.tile([128, 128], bf16)
        nc.sync.dma_start(idn[:], idn_e[:, :])
        mAl = const.tile([5, 128], f32)
        nc.sync.dma_start(mAl[:], mAl_e[:, :])
        mAr = const.tile([5, 512], f32)
        nc.sync.dma_start(mAr[:], mAr_e[:, :])
        mBl = const.tile([3, 64], f32)
        nc.sync.dma_start(mBl[:], mBl_e[:, :])
        mBr = const.tile([3, 256], f32)
        nc.sync.dma_start(mBr[:], mBr_e[:, :])

        for s in range(NSTRIP):
            # ---- x strip load (10 rows x 258 cols, bf16) ----
            xsA = xs_p.tile([128, 2580], bf16, tag="xsA")
            nc.sync.dma_start(xsA[:], xp_e[0:128, 8 * s:8 * s + 10, :])
            xsB = xs_p.tile([64, 2580], bf16, tag="xsB")
            nc.sync.dma_start(xsB[:], xp_e[128:192, 8 * s:8 * s + 10, :])

            qk = [None, None, None]
            v3 = acc_p.tile([128, 32 * 65], bf16, tag="v3")
            v4 = acc_p.tile([64, 32 * 65], bf16, tag="v4")
            # ones column for the softmax-denominator trick
            nc.vector.memset(
                v3[:].rearrange("p (w c) -> p w c", c=65)[:, :, 64:65], 1.0)
            nc.vector.memset(
                v4[0:64].rearrange("p (w c) -> p w c", c=65)[:, :, 64:65], 1.0)

            for m, (mo, msz) in enumerate(MCH):
                # ---- 1x1 qkv GEMM + bf16 evac ----
                q1t = q1_p.tile([128 if msz == 128 else 64, 2580], bf16,
                                tag=f"q1_{m}")
                for nt in range(6):
                    n0 = 430 * nt
                    ps = pmm.tile([128, 512], f32, tag="mm")
                    nc.tensor.matmul(ps[0:msz, 0:430], w1A[:, mo:mo + msz],
                                     xsA[:, n0:n0 + 430], start=True, stop=False)
                    nc.tensor.matmul(ps[0:msz, 0:430], w1B[:, mo:mo + msz],
                                     xsB[:, n0:n0 + 430], start=False, stop=True)
                    nc.scalar.copy(q1t[0:msz, n0:n0 + 430], ps[0:msz, 0:430])

                # ---- depthwise 3x3: out(r,c) = sum_t w[c,t]*q1(r+dy, c+dx) ----
                view = q1t[0:msz].rearrange("p (r c) -> p r c", r=10)

                def in0(t):
                    dy, dx = t // 3, t % 3
                    return view[:, dy:dy + 8, dx:dx + 256]

                def in0row(t, r):
                    dy, dx = t // 3, t % 3
                    off = 258 * (dy + r) + dx
                    return q1t[0:msz, off:off + 256].rearrange(
                        "p (w c) -> p w c", w=32)

                def wap(t):
                    return dwW[0:msz, 9 * m + t:9 * m + t + 1]

                def rm(tile_, n):
                    return tile_[0:n].rearrange("p (r c) -> p r c", r=8)

                def rowv(tile_, n, r):
                    return tile_[0:n, 256 * r:256 * r + 256].rearrange(
                        "p (w c) -> p w c", w=32)

                def vrow(vt, n, r):
                    # v tile row r as [p, 32 windows (stride 65), 8 cols]
                    return vt[0:n].rearrange("p (w c) -> p w c", c=65)[
                        :, :, 8 * r:8 * r + 8]

                if m < 3:
                    accm = acc_p.tile([128, 2048], bf16, tag=f"acc{m}")
                else:
                    vt = (v3, v4)[m - 3]

                if m == 3:
                    # GpSimd: tensor_scalar mult + tensor_tensor add chain
                    # (Pool has no scalar_tensor_tensor opcode)
                    cur = tap_p.tile([128, 2048], bf16, tag="pp0")
                    nc.gpsimd.tensor_scalar(rm(cur, msz), in0(0), wap(0),
                                            None, AL.mult)
                    for t in range(1, 9):
                        tmp = tap_p.tile([128, 2048], bf16, tag="pptmp")
                        nc.gpsimd.tensor_scalar(rm(tmp, msz), in0(t), wap(t),
                                                None, AL.mult)
                        if t == 8:
                            for r in range(8):
                                nc.gpsimd.tensor_tensor(
                                    vrow(vt, msz, r), rowv(tmp, msz, r),
                                    rowv(cur, msz, r), AL.add)
                        else:
                            nxt = tap_p.tile([128, 2048], bf16,
                                             tag=f"pp{t % 2}")
                            nc.gpsimd.tensor_tensor(rm(nxt, msz),
                                                    rm(tmp, msz),
                                                    rm(cur, msz), AL.add)
                            cur = nxt
                else:
                    # taps: DVE tensor_scalar mults (m==1: ACT mults instead)
                    taps = []
                    for t in range(9):
                        tp = tap_p.tile([128, 2048], bf16, tag=f"d{t}")
                        if m == 1:
                            nc.scalar.activation(rm(tp, msz), in0(t), AF.Copy,
                                                 scale=wap(t))
                        else:
                            nc.vector.tensor_scalar(rm(tp, msz), in0(t),
                                                    wap(t), None, AL.mult)
                        taps.append(tp)
                    # tree adds on DVE (flat, packed bf16 -> 2x mode)
                    lv1 = []
                    for i in range(4):
                        st = tap_p.tile([128, 2048], bf16, tag=f"s{i}")
                        nc.vector.tensor_tensor(
                            st[0:msz], taps[2 * i][0:msz],
                            taps[2 * i + 1][0:msz], AL.add)
                        lv1.append(st)
                    w0t = tap_p.tile([128, 2048], bf16, tag="w0")
                    nc.vector.tensor_tensor(w0t[0:msz], lv1[0][0:msz],
                                            lv1[1][0:msz], AL.add)
                    w1t = tap_p.tile([128, 2048], bf16, tag="w1")
                    nc.vector.tensor_tensor(w1t[0:msz], lv1[2][0:msz],
                                            lv1[3][0:msz], AL.add)
                    x0t = tap_p.tile([128, 2048], bf16, tag="x0")
                    nc.vector.tensor_tensor(x0t[0:msz], w0t[0:msz],
                                            w1t[0:msz], AL.add)
                    if m == 4:
                        for r in range(8):
                            nc.vector.tensor_tensor(
                                vrow(vt, msz, r), rowv(x0t, msz, r),
                                rowv(taps[8], msz, r), AL.add)
                    else:
                        nc.vector.tensor_tensor(accm[0:msz], x0t[0:msz],
                                                taps[8][0:msz], AL.add)

                # ---- q,k: per-(chan,window) l2 norm + fused 1/norm scale ----
                # acc is row-major [r, c]; window w covers cols 8w..8w+8 of
                # every row. Two-stage reduce keeps APs <= 3D.
                if m < 3:
                    sqt = sq_p.tile([128, 2048], bf16, tag="sq")
                    nc.scalar.activation(sqt[0:msz], accm[0:msz], AF.Square)
                    n2c = sm_p.tile([128, 256], f32, tag="n2c")
                    nc.vector.tensor_reduce(
                        n2c[0:msz],
                        sqt[0:msz].rearrange("p (r c) -> p c r", r=8),
                        axis=AX.X, op=AL.add)
                    n2 = sm_p.tile([128, 32], f32, tag="n2")
                    nc.vector.tensor_reduce(
                        n2[0:msz], n2c[0:msz].rearrange("p (w c) -> p w c",
                                                        w=32),
                        axis=AX.X, op=AL.add)
                    nrm = sm_p.tile([128, 32], f32, tag="nrm")
                    nc.scalar.activation(nrm[0:msz], n2[0:msz], AF.Sqrt)
                    rj = sm_p.tile([128, 32], f32, tag="rj")
                    nc.vector.reciprocal(rj[0:msz], nrm[0:msz])
                    # scale + regroup rows into windows (8 per-row STTs)
                    qkm = qk_p.tile([128, 2048], bf16, tag=f"qk{m}")
                    for r in range(8):
                        nc.vector.scalar_tensor_tensor(
                            qkm[0:msz].rearrange("p (w c) -> p w c", c=64)[
                                :, :, 8 * r:8 * r + 8],
                            rowv(accm, msz, r), 1.0,
                            rj[0:msz].unsqueeze(2).broadcast_to((msz, 32, 8)),
                            AL.mult, AL.mult)
                    qk[m] = qkm

            # ---- attention ----
            for half in range(2):
                if half == 0:
                    att_A = att_p.tile([128, 2048], bf16, tag="attA")
                    att_B = att_p.tile([64, 2048], bf16, tag="attB")
                qkpc = pc_p.tile([128, 8 * 384], bf16, tag="qkpc")
                for p8 in range(8):
                    w0 = 16 * half + 2 * p8
                    pcps = pct.tile([128, 384], bf16, tag="pct")
                    for (srt, oc) in ((qk[0], 0), (qk[1], 128), (qk[2], 256)):
                        nc.tensor.transpose(pcps[:, oc:oc + 128],
                                            srt[0:128, 64 * w0:64 * w0 + 128],
                                            idn[:, :])
                    nc.scalar.copy(qkpc[:, 384 * p8:384 * (p8 + 1)], pcps[:])
                for quad in range(4):
                    gA = pg.tile([128, 512], f32, tag="ga")
                    gB = pgB.tile([64, 256], f32, tag="gb")
                    nc.tensor.matmul(gA[:], mAl[:], mAr[:], start=True,
                                     stop=False, skip_group_check=True)
                    nc.tensor.matmul(gB[0:64, :], mBl[:], mBr[:], start=True,
                                     stop=False, skip_group_check=True)
                    for wq in range(4):
                        wl = 4 * quad + wq
                        p8i, wsub = wl // 2, wl % 2
                        pb = 64 * wsub
                        qa = qkpc[pb:pb + 64, 384 * p8i:384 * p8i + 128]
                        ka = qkpc[pb:pb + 64, 384 * p8i + 192:384 * p8i + 320]
                        # transposed gram: E' = exp(k^T q) has j on partitions
                        nc.tensor.matmul(gA[:, 128 * wq:128 * wq + 128], ka, qa,
                                         start=False, stop=(wq == 3),
                                         skip_group_check=True)
                        qb = qkpc[pb:pb + 64, 384 * p8i + 128:384 * p8i + 192]
                        kb = qkpc[pb:pb + 64, 384 * p8i + 320:384 * p8i + 384]
                        nc.tensor.matmul(gB[0:64, 64 * wq:64 * wq + 64], kb, qb,
                                         start=False, stop=(wq == 3),
                                         skip_group_check=True)
                    eA = e_p.tile([128, 512], bf16, tag="eA")
                    eB = e_p.tile([64, 256], bf16, tag="eB")
                    nc.scalar.activation(eA[:], gA[:], AF.Exp, scale=tmpA[:])
                    nc.scalar.activation(eB[0:64, :], gB[0:64, :], AF.Exp,
                                         scale=tmpB[:])
                    oA = po.tile([128, 260], f32, tag="oA")
                    oB = po.tile([64, 260], f32, tag="oB")
                    for wq in range(4):
                        wg = 16 * half + 4 * quad + wq
                        nc.tensor.matmul(oA[:, 65 * wq:65 * wq + 65],
                                         eA[:, 128 * wq:128 * wq + 128],
                                         v3[:, 65 * wg:65 * wg + 65],
                                         start=True, stop=True)
                        nc.tensor.matmul(oB[0:64, 65 * wq:65 * wq + 65],
                                         eB[0:64, 64 * wq:64 * wq + 64],
                                         v4[0:64, 65 * wg:65 * wg + 65],
                                         start=True, stop=True)
                    rzA = sm_p.tile([128, 4], f32, tag="rzA")
                    rzB = sm_p.tile([64, 4], f32, tag="rzB")
                    nc.vector.reciprocal(
                        rzA[:].unsqueeze(2),
                        oA[:].rearrange("p (w c) -> p w c", c=65)[:, :, 64:65])
                    nc.vector.reciprocal(
                        rzB[0:64].unsqueeze(2),
                        oB[0:64].rearrange("p (w c) -> p w c",
                                           c=65)[:, :, 64:65])
                    ob = 1024 * half + 256 * quad
                    nc.vector.scalar_tensor_tensor(
                        att_A[:, ob:ob + 256].rearrange("p (w c) -> p w c",
                                                        w=4),
                        oA[:].rearrange("p (w c) -> p w c", c=65)[:, :, 0:64],
                        1.0,
                        rzA[:].unsqueeze(2).broadcast_to((128, 4, 64)),
                        AL.mult, AL.mult)
                    nc.vector.scalar_tensor_tensor(
                        att_B[0:64, ob:ob + 256].rearrange("p (w c) -> p w c",
                                                           w=4),
                        oB[0:64].rearrange("p (w c) -> p w c",
                                           c=65)[:, :, 0:64],
                        1.0,
                        rzB[0:64].unsqueeze(2).broadcast_to((64, 4, 64)),
                        AL.mult, AL.mult)

            # ---- proj GEMM (restores flat pixel order) ----
            yA = y_p.tile([128, 2048], f32, tag="yA")
            yB = y_p.tile([64, 2048], f32, tag="yB")
            attAr = att_A[:, :].rearrange("p (w r c) -> p r w c", w=32, r=8)
            attBr = att_B[0:64, :].rearrange("p (w r c) -> p r w c", w=32, r=8)
            for nb in range(4):
                ppA = pmm.tile([128, 512], f32, tag="mm")
                ppB = pgB.tile([64, 512], f32, tag="gb2")
                for mo2, msz2, pp in ((0, 128, ppA), (128, 64, ppB)):
                    for kc in range(2):
                        lhsT = (pjA, pjB)[kc][:, mo2:mo2 + msz2]
                        rhs = (attAr, attBr)[kc][:, 2 * nb:2 * nb + 2, :, :]
                        nc.tensor.matmul(pp[0:msz2, :], lhsT, rhs,
                                         start=(kc == 0), stop=(kc == 1))
                nc.vector.tensor_copy(yA[:, 512 * nb:512 * (nb + 1)], ppA[:])
                nc.scalar.copy(yB[0:64, 512 * nb:512 * (nb + 1)], ppB[0:64, :])
            nc.sync.dma_start(y_e[0:128, 8 * s:8 * s + 8, :], yA[:])
            nc.sync.dma_start(y_e[128:192, 8 * s:8 * s + 8, :], yB[0:64, :])

    nc.finalize()
    return nc


def kernel(**inputs):
    global LAST_RESULT
    from concourse.bass_utils import run_bass_kernel_spmd

    x = np.ascontiguousarray(np.asarray(inputs["x"], dtype=np.float32))
    qkv_w = np.asarray(inputs["qkv_w"], dtype=np.float32)[:, :, 0, 0]
    dw_w = np.asarray(inputs["dw_w"], dtype=np.float32)[:, 0]
    proj_w = np.asarray(inputs["proj_w"], dtype=np.float32)[:, :, 0, 0]
    temp = np.asarray(inputs["temperature"], dtype=np.float32)[:, 0, 0]

    w1T = qkv_w.T  # [192 in, 576 out]
    w1A = np.ascontiguousarray(w1T[0:128, :]).astype(BF16)
    w1B = np.ascontiguousarray(w1T[128:192, :]).astype(BF16)
    dwW = np.zeros((128, 45), np.float32)
    for m, (mo, msz) in enumerate(MCH):
        for t in range(9):
            dwW[0:msz, 9 * m + t] = dw_w[mo:mo + msz, t // 3, t % 3]
    projT = np.ascontiguousarray(proj_w.T)
    pjA = np.ascontiguousarray(projT[0:128, :]).astype(BF16)
    pjB = np.ascontiguousarray(projT[128:192, :]).astype(BF16)
    tmpA = np.repeat(temp[0:4], 32).reshape(128, 1).astype(np.float32)
    tmpB = np.repeat(temp[4:6], 32).reshape(64, 1).astype(np.float32)
    idn = np.eye(128, dtype=np.float32).astype(BF16)
    mAl = np.zeros((5, 128), np.float32)
    mAr = np.zeros((5, 512), np.float32)
    mAl[0, :] = 10.0
    mAr[0, :] = -10.0
    for g in range(4):
        mAl[1 + g, 32 * g:32 * g + 32] = 10.0
        for rep in range(4):
            mAr[1 + g, 128 * rep + 32 * g:128 * rep + 32 * g + 32] = 10.0
    mBl = np.zeros((3, 64), np.float32)
    mBr = np.zeros((3, 256), np.float32)
    mBl[0, :] = 10.0
    mBr[0, :] = -10.0
    for g in range(2):
        mBl[1 + g, 32 * g:32 * g + 32] = 10.0
        for rep in range(4):
            mBr[1 + g, 64 * rep + 32 * g:64 * rep + 32 * g + 32] = 10.0

    xpad = np.pad(x, ((0, 0), (0, 0), (1, 1), (1, 1)))
    in_maps = []
    for core in range(NCORES):
        b, halfc = core // 2, core % 2
        r0 = 128 * halfc
        xp = np.ascontiguousarray(xpad[b, :, r0:r0 + 130, :]).astype(BF16)
        in_maps.append(dict(xp=xp, w1A=w1A, w1B=w1B, dwW=dwW, pjA=pjA,
                            pjB=pjB, tmpA=tmpA, tmpB=tmpB, idn=idn, mAl=mAl,
                            mAr=mAr, mBl=mBl, mBr=mBr))

    if "nc" not in _CACHE:
        _CACHE["nc"] = _build_nc()
    nc = _CACHE["nc"]

    res = run_bass_kernel_spmd(nc, in_maps, list(range(NCORES)))
    LAST_RESULT = res

    out = np.zeros((B, DIM, H, W), np.float32)
    for core in range(NCORES):
        b, halfc = core // 2, core % 2
        r0 = 128 * halfc
        out[b, :, r0:r0 + 128, :] = res.results[core]["y"]
    return out
